# revision 1
# baseline (speedup 1.0000x reference)
"""Mamba block kernel, 4-core batch-parallel (cores 0-3, one batch each).

Optimized for the emulated-NRT backend where per-instruction fixed cost
dominates (~25us DVE / ~87us ACT / ~52-87us matmul / ~20us DMA) and
cross-core parallelism saturates at ~4 streams: one full sequence per
core, zero collectives, fp32 everywhere (bf16 is slower under emulation),
maximum-size tiles (scan as one [128, 16*512] DVE instruction per
(j-tile, chunk)).

Pipeline per core: in_proj x -> clip -> depthwise conv (DVE shifts) ->
silu+clip -> u (spill DRAM) + x_proj PSUM accumulation; in_proj z ->
clip -> silu -> g (spill DRAM); dt_proj + softplus; selective scan via
segmented tensor_tensor_scan with n in the free dim (segment isolation
by zeroing the decay column at t=0 of each chunk and injecting the
carry into b's first column); y = clip(sum_n C*h + u*D) * g; out_proj.
"""
import sys
sys.path.insert(0, "/opt/trn_rl_repo")
import numpy as np
import concourse.bass as bass
import concourse.bacc as bacc
import concourse.mybir as mybir
from concourse.tile import TileContext
from concourse.bass_utils import run_bass_kernel_spmd

F32 = mybir.dt.float32
OP = mybir.AluOpType
AF = mybir.ActivationFunctionType

B_, L, DM = 4, 2048, 1024
DI = 2048
N = 16
RK = 64
KC = 4
NJ = DI // 128        # 16 d_inner tiles
NK = DM // 128        # 8 k tiles over d_model
NM = DM // 128        # 8 out tiles
TC = 512
NCH = L // TC         # 4 chunks

_CACHED_NC = {}


def _build(reps=1):
    nc = bacc.Bacc(num_devices=4)

    hst = nc.declare_dram_parameter("hst", [DM, L], F32, isOutput=False)
    wxT = nc.declare_dram_parameter("wxT", [DM, DI], F32, isOutput=False)
    wzT = nc.declare_dram_parameter("wzT", [DM, DI], F32, isOutput=False)
    convw = nc.declare_dram_parameter("convw", [128, NJ, KC], F32, isOutput=False)
    convb = nc.declare_dram_parameter("convb", [128, NJ], F32, isOutput=False)
    wxpT = nc.declare_dram_parameter("wxpT", [DI, RK + 2 * N], F32, isOutput=False)
    wdtT = nc.declare_dram_parameter("wdtT", [RK, DI], F32, isOutput=False)
    bdt = nc.declare_dram_parameter("bdt", [128, NJ], F32, isOutput=False)
    negA = nc.declare_dram_parameter("negA", [128, NJ, N], F32, isOutput=False)
    dvec = nc.declare_dram_parameter("dvec", [128, NJ], F32, isOutput=False)
    woT = nc.declare_dram_parameter("woT", [DI, DM], F32, isOutput=False)
    oslab = nc.declare_dram_parameter("oslab", [DM, L], F32, isOutput=True)

    P = dict(hst=hst, wxT=wxT, wzT=wzT, convw=convw, convb=convb, wxpT=wxpT,
             wdtT=wdtT, bdt=bdt, negA=negA, dvec=dvec, woT=woT, oslab=oslab)

    with TileContext(nc) as tc:
        with tc.tile_pool(name="const", bufs=1) as cp:
            C = {}
            C["bdt"] = cp.tile([128, NJ], F32, tag="bdt", name="bdt_t")
            nc.sync.dma_start(out=C["bdt"][:, :], in_=bdt[:, :])
            C["negA"] = cp.tile([128, NJ, N], F32, tag="negA", name="negA_t")
            nc.sync.dma_start(out=C["negA"][:, :, :], in_=negA[:, :, :])
            C["dvec"] = cp.tile([128, NJ], F32, tag="dvec", name="dvec_t")
            nc.sync.dma_start(out=C["dvec"][:, :], in_=dvec[:, :])
            C["wdtT"] = cp.tile([RK, DI], F32, tag="wdtT", name="wdtT_t")
            nc.sync.dma_start(out=C["wdtT"][:, :], in_=wdtT[:, :])

            for rep in range(reps):
                D_ = {
                    "ug_dram": nc.dram_tensor(f"ug_dram{rep}", [DI, 2, L], F32),
                    "bc_dram": nc.dram_tensor(f"bc_dram{rep}", [2 * N, L], F32),
                }
                _emit(nc, tc, P, C, D_, rep)

    nc.finalize()
    return nc


def _emit(nc, tc, P, C, D_, rep):
    # ---------------- phase 1 + 2: projections, conv, u/g, x_dbl ----------
    with tc.tile_pool(name=f"keep{rep}", bufs=1) as kp:
        dtraw = kp.tile([RK, L], F32, tag="dtraw")
        carries = kp.tile([128, NJ, N], F32, tag="carries")

        with (
            tc.tile_pool(name=f"xd{rep}", bufs=1) as xdp,
            tc.tile_pool(name=f"psx{rep}", bufs=NCH, space="PSUM") as psxp,
            tc.tile_pool(name=f"hs{rep}", bufs=1) as hp,
            tc.tile_pool(name=f"w1{rep}", bufs=2) as wp,
            tc.tile_pool(name=f"xc{rep}", bufs=2) as xcp,
            tc.tile_pool(name=f"cv{rep}", bufs=1) as cvp,
            tc.tile_pool(name=f"u1{rep}", bufs=2) as up,
            tc.tile_pool(name=f"ps1{rep}", bufs=2, space="PSUM") as psp,
        ):
            xdbl = xdp.tile([RK + 2 * N, L], F32, tag="xdbl")
            psx = [psxp.tile([RK + 2 * N, TC], F32, tag="psx", name=f"psx{c}")
                   for c in range(NCH)]
            hs = hp.tile([128, NK, L], F32, tag="hs")
            nc.sync.dma_start(
                out=hs[:, :, :],
                in_=P["hst"][:, :].rearrange("(k p) t -> p k t", k=NK))
            cw = hp.tile([128, NJ, KC], F32, tag="cw")
            nc.sync.dma_start(out=cw[:, :, :], in_=P["convw"][:, :, :])
            cb = hp.tile([128, NJ], F32, tag="cb")
            nc.sync.dma_start(out=cb[:, :], in_=P["convb"][:, :])
            wxp = hp.tile([128, NJ, RK + 2 * N], F32, tag="wxp")
            nc.sync.dma_start(
                out=wxp[:, :, :],
                in_=P["wxpT"][:, :].rearrange("(j p) w -> p j w", j=NJ))

            # ---- x half of in_proj + conv + u + x_proj accumulation ----
            for j in range(NJ):
                wt = wp.tile([128, NK, 128], F32, tag="w_in")
                nc.sync.dma_start(
                    out=wt[:, :, :],
                    in_=P["wxT"][:, j * 128:(j + 1) * 128].rearrange(
                        "(k p) q -> p k q", k=NK))
                xc = xcp.tile([128, KC - 1 + L], F32, tag="xc")
                nc.vector.memset(xc[:, 0:KC - 1], 0.0)
                for c in range(NCH):
                    ps = psp.tile([128, TC], F32, tag="ps")
                    for k in range(NK):
                        nc.tensor.matmul(
                            ps[:, :], wt[:, k, :], hs[:, k, c * TC:(c + 1) * TC],
                            start=(k == 0), stop=(k == NK - 1))
                    nc.vector.tensor_scalar(
                        xc[:, KC - 1 + c * TC: KC - 1 + (c + 1) * TC],
                        ps[:, :], 0.0, 1.0, op0=OP.max, op1=OP.min)
                # depthwise conv: tap products into planes, one strided reduce
                tp4 = cvp.tile([128, KC, L], F32, tag="tp4")
                for k in range(KC):
                    nc.vector.tensor_tensor(
                        out=tp4[:, k, :], in0=xc[:, k:k + L],
                        in1=cw[:, j, k:k + 1].broadcast_to([128, L]),
                        op=OP.mult)
                ca = cvp.tile([128, L], F32, tag="ca")
                nc.vector.tensor_reduce(
                    out=ca[:, :], in_=tp4.rearrange("p k t -> p t k"),
                    axis=mybir.AxisListType.X, op=OP.add)
                us = up.tile([128, L], F32, tag="us")
                nc.scalar.activation(us[:, :], ca[:, :], AF.Silu,
                                     bias=cb[:, j:j + 1])
                u = up.tile([128, L], F32, tag="u")
                nc.vector.tensor_scalar(u[:, :], us[:, :], 0.0, 1.0,
                                        op0=OP.max, op1=OP.min)
                nc.sync.dma_start(
                    out=D_["ug_dram"][j * 128:(j + 1) * 128, 0, :], in_=u[:, :])
                for c in range(NCH):
                    nc.tensor.matmul(
                        psx[c][:, :], wxp[:, j, :],
                        u[:, c * TC:(c + 1) * TC],
                        start=(j == 0), stop=(j == NJ - 1))

            # ---- z half of in_proj -> gate g ----
            for j in range(NJ):
                wt = wp.tile([128, NK, 128], F32, tag="w_in")
                nc.sync.dma_start(
                    out=wt[:, :, :],
                    in_=P["wzT"][:, j * 128:(j + 1) * 128].rearrange(
                        "(k p) q -> p k q", k=NK))
                zf = up.tile([128, L], F32, tag="us")
                for c in range(NCH):
                    ps = psp.tile([128, TC], F32, tag="ps")
                    for k in range(NK):
                        nc.tensor.matmul(
                            ps[:, :], wt[:, k, :], hs[:, k, c * TC:(c + 1) * TC],
                            start=(k == 0), stop=(k == NK - 1))
                    nc.vector.tensor_scalar(
                        zf[:, c * TC:(c + 1) * TC], ps[:, :], 0.0, 1.0,
                        op0=OP.max, op1=OP.min)
                g = up.tile([128, L], F32, tag="u")
                nc.scalar.activation(g[:, :], zf[:, :], AF.Silu)
                nc.sync.dma_start(
                    out=D_["ug_dram"][j * 128:(j + 1) * 128, 1, :], in_=g[:, :])

            # ---- x_dbl out of PSUM; B/C rows to DRAM for broadcast ----
            for c in range(NCH):
                nc.scalar.copy(xdbl[:, c * TC:(c + 1) * TC], psx[c][:, :])
            nc.vector.tensor_scalar(dtraw[:, :], xdbl[0:RK, :], 0.0, 1.0,
                                    op0=OP.max, op1=OP.min)
            nc.sync.dma_start(out=D_["bc_dram"][:, :],
                              in_=xdbl[RK:RK + 2 * N, :])

        # ---------------- phase 3: dt, scan, gate, out_proj --------------
        with (
            tc.tile_pool(name=f"bc{rep}", bufs=1) as bcp,
            tc.tile_pool(name=f"ab{rep}", bufs=1) as abp,
            tc.tile_pool(name=f"yb{rep}", bufs=1) as ybp,
            tc.tile_pool(name=f"wk{rep}", bufs=1) as wkp,
            tc.tile_pool(name=f"wo{rep}", bufs=1) as wop,
            tc.tile_pool(name=f"ps3{rep}", bufs=2, space="PSUM") as ps3,
            tc.tile_pool(name=f"pso{rep}", bufs=2, space="PSUM") as pso,
        ):
            for c in range(NCH):
                csl = slice(c * TC, (c + 1) * TC)
                Bc = bcp.tile([128, N, TC], F32, tag="Bc")
                nc.sync.dma_start(
                    out=Bc[:, :, :],
                    in_=D_["bc_dram"][None, 0:N, csl].broadcast_to([128, N, TC]))
                Cc = bcp.tile([128, N, TC], F32, tag="Cc")
                nc.sync.dma_start(
                    out=Cc[:, :, :],
                    in_=D_["bc_dram"][None, N:2 * N, csl].broadcast_to(
                        [128, N, TC]))
                yblk = ybp.tile([128, NJ, TC], F32, tag="yblk")

                for j in range(NJ):
                    # dt_proj -> softplus -> clip
                    psd = ps3.tile([128, TC], F32, tag="psd")
                    nc.tensor.matmul(
                        psd[:, :], C["wdtT"][:, j * 128:(j + 1) * 128],
                        dtraw[:, csl], start=True, stop=True)
                    spe = wkp.tile([128, TC], F32, tag="spe")
                    nc.scalar.activation(spe[:, :], psd[:, :], AF.Exp,
                                         bias=C["bdt"][:, j:j + 1])
                    dt = wkp.tile([128, TC], F32, tag="dt")
                    nc.scalar.activation(dt[:, :], spe[:, :], AF.Ln, bias=1.0)
                    nc.vector.tensor_scalar(dt[:, :], dt[:, :], 1e-4, 20.0,
                                            op0=OP.max, op1=OP.min)
                    # u / g for this (j, c): one combined load
                    ug = wkp.tile([128, 2, TC], F32, tag="ug", bufs=2)
                    nc.gpsimd.dma_start(
                        out=ug[:, :, :],
                        in_=D_["ug_dram"][j * 128:(j + 1) * 128, :, csl])
                    u = ug[:, 0, :]
                    g = ug[:, 1, :]
                    dtu = wkp.tile([128, TC], F32, tag="dtu")
                    nc.vector.tensor_tensor(out=dtu[:, :], in0=dt[:, :],
                                            in1=u[:, :], op=OP.mult)
                    # a = exp(dt * -A) over [128, N, TC]
                    a3 = abp.tile([128, N, TC], F32, tag="a3")
                    nc.vector.tensor_tensor(
                        out=a3[:, :, :],
                        in0=dt[:, None, :].broadcast_to([128, N, TC]),
                        in1=C["negA"][:, j, :, None].broadcast_to([128, N, TC]),
                        op=OP.mult)
                    nc.scalar.activation(
                        a3.rearrange("p n t -> p (n t)"),
                        a3.rearrange("p n t -> p (n t)"), AF.Exp)
                    # b = dtu * B
                    b3 = abp.tile([128, N, TC], F32, tag="b3")
                    nc.vector.tensor_tensor(
                        out=b3[:, :, :],
                        in0=dtu[:, None, :].broadcast_to([128, N, TC]),
                        in1=Bc[:, :, :], op=OP.mult)
                    # segment boundary: b[:, :, 0] += a[:, :, 0]*carry; a[:, :, 0] = 0
                    # (at c == 0 the carry is all-zero, so the inject is a no-op)
                    if c > 0:
                        tmp0 = wkp.tile([128, N, 1], F32, tag="tmp0")
                        nc.vector.tensor_tensor(
                            out=tmp0[:, :, :], in0=a3[:, :, 0:1],
                            in1=carries[:, j, :, None], op=OP.mult)
                        nc.vector.tensor_tensor(
                            out=b3[:, :, 0:1], in0=b3[:, :, 0:1],
                            in1=tmp0[:, :, :], op=OP.add)
                    nc.vector.memset(a3[:, :, 0:1], 0.0)
                    # scan (in place into b3)
                    nc.vector.tensor_tensor_scan(
                        b3.rearrange("p n t -> p (n t)"),
                        a3.rearrange("p n t -> p (n t)"),
                        b3.rearrange("p n t -> p (n t)"),
                        0.0, op0=OP.mult, op1=OP.add)
                    if c < NCH - 1:
                        nc.scalar.copy(carries[:, j, :],
                                       b3[:, :, TC - 1])
                    # ch = h * C, reduce over n (into a3)
                    nc.vector.tensor_tensor(out=a3[:, :, :], in0=b3[:, :, :],
                                            in1=Cc[:, :, :], op=OP.mult)
                    red = wkp.tile([128, TC], F32, tag="red")
                    nc.vector.tensor_reduce(
                        out=red[:, :], in_=a3.rearrange("p n t -> p t n"),
                        axis=mybir.AxisListType.X, op=OP.add)
                    # y = clip(red + u*D) * g
                    uD = wkp.tile([128, TC], F32, tag="spe")
                    nc.vector.tensor_tensor(
                        out=uD[:, :], in0=u[:, :],
                        in1=C["dvec"][:, j:j + 1].broadcast_to([128, TC]),
                        op=OP.mult)
                    yt = wkp.tile([128, TC], F32, tag="dtu")
                    nc.vector.tensor_tensor(out=yt[:, :], in0=red[:, :],
                                            in1=uD[:, :], op=OP.add)
                    nc.vector.tensor_scalar(yt[:, :], yt[:, :], 0.0, 1.0,
                                            op0=OP.max, op1=OP.min)
                    nc.vector.tensor_tensor(out=yblk[:, j, :], in0=yt[:, :],
                                            in1=g[:, :], op=OP.mult)

                # out_proj for this chunk
                for m in range(NM):
                    wo = wop.tile([128, NJ, 128], F32, tag="wo")
                    nc.sync.dma_start(
                        out=wo[:, :, :],
                        in_=P["woT"][:, m * 128:(m + 1) * 128].rearrange(
                            "(j p) q -> p j q", j=NJ))
                    po = pso.tile([128, TC], F32, tag="po")
                    for j in range(NJ):
                        nc.tensor.matmul(
                            po[:, :], wo[:, j, :], yblk[:, j, :],
                            start=(j == 0), stop=(j == NJ - 1))
                    ob = wkp.tile([128, TC], F32, tag="ob", bufs=2)
                    nc.scalar.copy(ob[:, :], po[:, :])
                    nc.sync.dma_start(
                        out=P["oslab"][m * 128:(m + 1) * 128, csl], in_=ob[:, :])


def _shard(inputs):
    hs = np.asarray(inputs["hidden_states"], np.float32)
    W_in = np.asarray(inputs["W_in"], np.float32)
    conv_w = np.asarray(inputs["conv_w"], np.float32)
    conv_b = np.asarray(inputs["conv_b"], np.float32)
    W_x = np.asarray(inputs["W_x"], np.float32)
    W_dt = np.asarray(inputs["W_dt"], np.float32)
    b_dt = np.asarray(inputs["b_dt"], np.float32)
    W_out = np.asarray(inputs["W_out"], np.float32)
    A_log = np.asarray(inputs["A_log"], np.float32)
    D = np.asarray(inputs["D"], np.float32)

    negA = (-np.exp(A_log)).reshape(NJ, 128, N).transpose(1, 0, 2)
    common = {
        "wxT": np.ascontiguousarray(W_in[:DI].T),
        "wzT": np.ascontiguousarray(W_in[DI:].T),
        "convw": np.ascontiguousarray(
            conv_w[:, 0, :].reshape(NJ, 128, KC).transpose(1, 0, 2)),
        "convb": np.ascontiguousarray(conv_b.reshape(NJ, 128).T),
        "wxpT": np.ascontiguousarray(W_x.T),
        "wdtT": np.ascontiguousarray(W_dt.T),
        "bdt": np.ascontiguousarray(b_dt.reshape(NJ, 128).T),
        "negA": np.ascontiguousarray(negA),
        "dvec": np.ascontiguousarray(D.reshape(NJ, 128).T),
        "woT": np.ascontiguousarray(W_out.T),
    }
    in_maps = []
    for b in range(4):
        m = dict(common)
        m["hst"] = np.ascontiguousarray(hs[b].T)
        in_maps.append(m)
    return in_maps


def kernel(**inputs):
    if 1 not in _CACHED_NC:
        _CACHED_NC[1] = _build(1)
    nc = _CACHED_NC[1]
    in_maps = _shard(inputs)
    res = run_bass_kernel_spmd(nc, in_maps, core_ids=list(range(4)))
    out = np.empty((B_, L, DM), np.float32)
    for b in range(B_):
        out[b] = res.results[b]["oslab"].T
    return out



# revision 18
# speedup vs baseline: 2.3943x; 2.3943x over previous
"""Mamba block kernel, 8-core tensor-parallel (2 cores per batch, d_inner
split in halves, as in Mamba TP).

Targeting the emulated-NRT backend: per-instruction fixed cost (~40-60us)
dominates and each core's stream executes serially, while separate cores
overlap well on marginal reps. So: minimize per-core instruction count.
Per core (~1.5k instructions vs 3.9k in the 4-core batch-parallel layout):
in_proj/conv/x_proj/dt_proj/scan/out_proj over the local 8 j-tiles
(d_inner half), elementwise work batched into whole-[128, 8*512] tiles,
one pair-AllReduce for x_proj partials and per-chunk bf16 pair-AllReduce
for out_proj partials. dt_proj bias is folded into the matmul via an
extra contraction row so softplus-exp needs no per-j bias and can run on
j-pairs.
"""
import sys
sys.path.insert(0, "/opt/trn_rl_repo")
import numpy as np
import concourse.bass as bass
import concourse.bacc as bacc
import concourse.mybir as mybir
from concourse.tile import TileContext
from concourse.bass_utils import run_bass_kernel_spmd

F32 = mybir.dt.float32
BF16 = mybir.dt.bfloat16
OP = mybir.AluOpType
AF = mybir.ActivationFunctionType

B_, L, DM = 4, 2048, 1024
DI = 2048
DIH = DI // 2         # 1024 per core
N = 16
RK = 64
KC = 4
NJ = DIH // 128       # 8 local d_inner tiles
NK = DM // 128        # 8 k tiles over d_model
NM = DM // 128        # 8 out tiles
TC = 512
NCH = L // TC         # 4 chunks
GROUPS = [[0, 1], [2, 3], [4, 5], [6, 7]]

_CACHED_NC = {}


def _build(reps=1):
    nc = bacc.Bacc(num_devices=8)

    hst = nc.declare_dram_parameter("hst", [DM, L], F32, isOutput=False)
    wxT = nc.declare_dram_parameter("wxT", [DM, DIH], F32, isOutput=False)
    wzT = nc.declare_dram_parameter("wzT", [DM, DIH], F32, isOutput=False)
    convw = nc.declare_dram_parameter("convw", [128, NJ, KC], F32, isOutput=False)
    convb = nc.declare_dram_parameter("convb", [128, NJ], F32, isOutput=False)
    wxpT = nc.declare_dram_parameter("wxpT", [DIH, RK + 2 * N], F32, isOutput=False)
    wdtT = nc.declare_dram_parameter("wdtT", [RK + 1, DIH], F32, isOutput=False)
    negA = nc.declare_dram_parameter("negA", [128, NJ, N], F32, isOutput=False)
    dvec = nc.declare_dram_parameter("dvec", [128, NJ], F32, isOutput=False)
    woT = nc.declare_dram_parameter("woT", [DIH, DM], F32, isOutput=False)
    oslab = nc.declare_dram_parameter("oslab", [DM, L], BF16, isOutput=True)

    P = dict(hst=hst, wxT=wxT, wzT=wzT, convw=convw, convb=convb, wxpT=wxpT,
             wdtT=wdtT, negA=negA, dvec=dvec, woT=woT, oslab=oslab)

    with TileContext(nc) as tc:
        with tc.tile_pool(name="const", bufs=1) as cp:
            C = {}
            C["negA"] = cp.tile([128, NJ, N], F32, tag="negA", name="negA_t")
            nc.sync.dma_start(out=C["negA"][:, :, :], in_=negA[:, :, :])
            C["dvec"] = cp.tile([128, NJ], F32, tag="dvec", name="dvec_t")
            nc.sync.dma_start(out=C["dvec"][:, :], in_=dvec[:, :])
            C["wdtT"] = cp.tile([RK + 1, DIH], F32, tag="wdtT", name="wdtT_t")
            nc.sync.dma_start(out=C["wdtT"][:, :], in_=wdtT[:, :])
            C["convw"] = cp.tile([128, NJ, KC], F32, tag="convw", name="convw_t")
            nc.sync.dma_start(out=C["convw"][:, :, :], in_=convw[:, :, :])
            C["convb"] = cp.tile([128, NJ], F32, tag="convb", name="convb_t")
            nc.sync.dma_start(out=C["convb"][:, :], in_=convb[:, :])

            for rep in range(reps):
                D_ = {
                    "u_dram": nc.dram_tensor(f"u_dram{rep}", [DIH, L], F32),
                    "g_dram": nc.dram_tensor(f"g_dram{rep}", [DIH, L], F32),
                    "xdbl_dram": nc.dram_tensor(f"xdbl_dram{rep}",
                                                [RK + 2 * N, L], F32),
                    "outp_dram": [nc.dram_tensor(f"outp_dram{rep}_{c}",
                                                 [DM, TC], BF16)
                                  for c in range(NCH)],
                }
                _emit(nc, tc, P, C, D_, rep)

    nc.finalize()
    return nc


def _emit(nc, tc, P, C, D_, rep):
    # ================= phase A: in_proj, conv, u/g, x_proj ================
    with tc.tile_pool(name=f"keep{rep}", bufs=1) as kp:
        dtraw = kp.tile([RK + 1, L], F32, tag="dtraw")
        carries = kp.tile([128, NJ, N], F32, tag="carries")

        with tc.tile_pool(name=f"hs{rep}", bufs=1) as hp:
            hs = hp.tile([128, NK, L], F32, tag="hs")
            nc.sync.dma_start(
                out=hs[:, :, :],
                in_=P["hst"][:, :].rearrange("(k p) t -> p k t", k=NK))

            # ---- x half: in_proj + conv + silu + clip -> u; x_proj accum --
            with (
                tc.tile_pool(name=f"w1{rep}", bufs=2) as wp,
                tc.tile_pool(name=f"xc{rep}", bufs=1) as xcp,
                tc.tile_pool(name=f"cv{rep}", bufs=1) as cvp,
                tc.tile_pool(name=f"u1{rep}", bufs=2) as up,
                tc.tile_pool(name=f"psA{rep}", bufs=1, space="PSUM") as psa,
                tc.tile_pool(name=f"psXP{rep}", bufs=1, space="PSUM") as psxp,
            ):
                wxp = xcp.tile([128, NJ, RK + 2 * N], F32, tag="wxp")
                nc.sync.dma_start(
                    out=wxp[:, :, :],
                    in_=P["wxpT"][:, :].rearrange("(j p) w -> p j w", j=NJ))
                xp = psxp.tile([RK + 2 * N, NCH, TC], F32, tag="xp")
                xc = xcp.tile([128, KC - 1 + L], F32, tag="xc")
                nc.vector.memset(xc[:, 0:KC - 1], 0.0)

                for j in range(NJ):
                    wt = wp.tile([128, NK, 128], F32, tag="w_in")
                    nc.sync.dma_start(
                        out=wt[:, :, :],
                        in_=P["wxT"][:, j * 128:(j + 1) * 128].rearrange(
                            "(k p) q -> p k q", k=NK))
                    ps = psa.tile([128, NCH, TC], F32, tag="psA")
                    for c in range(NCH):
                        for k in range(NK):
                            nc.tensor.matmul(
                                ps[:, c, :], wt[:, k, :],
                                hs[:, k, c * TC:(c + 1) * TC],
                                start=(k == 0), stop=(k == NK - 1))
                    nc.vector.tensor_scalar(
                        xc[:, KC - 1:], ps.rearrange("p c t -> p (c t)"),
                        0.0, 1.0, op0=OP.max, op1=OP.min)
                    tp4 = cvp.tile([128, KC, L], F32, tag="tp4")
                    for k in range(KC):
                        nc.vector.tensor_tensor(
                            out=tp4[:, k, :], in0=xc[:, k:k + L],
                            in1=C["convw"][:, j, k:k + 1].broadcast_to([128, L]),
                            op=OP.mult)
                    ca = cvp.tile([128, L], F32, tag="ca")
                    nc.vector.tensor_reduce(
                        out=ca[:, :], in_=tp4.rearrange("p k t -> p t k"),
                        axis=mybir.AxisListType.X, op=OP.add)
                    us = up.tile([128, L], F32, tag="us")
                    nc.scalar.activation(us[:, :], ca[:, :], AF.Silu,
                                         bias=C["convb"][:, j:j + 1])
                    u = up.tile([128, L], F32, tag="u")
                    nc.vector.tensor_scalar(u[:, :], us[:, :], 0.0, 1.0,
                                            op0=OP.max, op1=OP.min)
                    nc.sync.dma_start(
                        out=D_["u_dram"][j * 128:(j + 1) * 128, :], in_=u[:, :])
                    for c in range(NCH):
                        nc.tensor.matmul(
                            xp[:, c, :], wxp[:, j, :], u[:, c * TC:(c + 1) * TC],
                            start=(j == 0), stop=(j == NJ - 1))

                # ---- x_dbl partial out of PSUM -> DRAM -> pair AllReduce --
                xps = up.tile([RK + 2 * N, L], F32, tag="us", name="xps")
                nc.vector.tensor_scalar(
                    xps[:, :], xp.rearrange("p c t -> p (c t)"),
                    0.0, 1.0, op0=OP.add, op1=OP.mult)
                nc.sync.dma_start(out=D_["xdbl_dram"][:, :], in_=xps[:, :])
                nc.gpsimd.collective_compute(
                    "AllReduce", OP.add, replica_groups=GROUPS,
                    ins=[D_["xdbl_dram"][:, :].opt()],
                    outs=[D_["xdbl_dram"][:, :].opt()])

            # ---- z half: in_proj + silu -> g, j-pairs on 8 psum banks ----
            with (
                tc.tile_pool(name=f"w2{rep}", bufs=2) as wp2,
                tc.tile_pool(name=f"zf{rep}", bufs=1) as zfp,
                tc.tile_pool(name=f"psZ{rep}", bufs=1, space="PSUM") as psz,
            ):
                for jp in range(NJ // 2):
                    pz = psz.tile([128, 2, NCH, TC], F32, tag="psZ")
                    for jj in range(2):
                        j = jp * 2 + jj
                        wt = wp2.tile([128, NK, 128], F32, tag="w_in2")
                        nc.sync.dma_start(
                            out=wt[:, :, :],
                            in_=P["wzT"][:, j * 128:(j + 1) * 128].rearrange(
                                "(k p) q -> p k q", k=NK))
                        for c in range(NCH):
                            for k in range(NK):
                                nc.tensor.matmul(
                                    pz[:, jj, c, :], wt[:, k, :],
                                    hs[:, k, c * TC:(c + 1) * TC],
                                    start=(k == 0), stop=(k == NK - 1))
                    zf = zfp.tile([128, 2, L], F32, tag="zf")
                    nc.vector.tensor_scalar(
                        zf.rearrange("p j t -> p (j t)"),
                        pz.rearrange("p j c t -> p (j c t)"),
                        0.0, 1.0, op0=OP.max, op1=OP.min)
                    g2 = zfp.tile([128, 2, L], F32, tag="g2")
                    nc.scalar.activation(
                        g2.rearrange("p j t -> p (j t)"),
                        zf.rearrange("p j t -> p (j t)"), AF.Silu)
                    nc.sync.dma_start(
                        out=D_["g_dram"][jp * 256:(jp + 1) * 256, :].rearrange(
                            "(j p) t -> p j t", j=2),
                        in_=g2[:, :, :])

            # ---- dtraw = clip(x_dbl[0:RK]) after the AllReduce; ones row --
            nc.sync.dma_start(out=dtraw[0:RK, :], in_=D_["xdbl_dram"][0:RK, :])
            nc.vector.tensor_scalar(dtraw[0:RK, :], dtraw[0:RK, :], 0.0, 1.0,
                                    op0=OP.max, op1=OP.min)
            nc.vector.memset(dtraw[RK:RK + 1, :], 1.0)

        # ================= phase B: dt, scan, gate, out_proj ==============
        with (
            tc.tile_pool(name=f"bc{rep}", bufs=1) as bcp,
            tc.tile_pool(name=f"ab{rep}", bufs=1) as abp,
            tc.tile_pool(name=f"wk{rep}", bufs=1) as wkp,
            tc.tile_pool(name=f"wo{rep}", bufs=1) as wop,
            tc.tile_pool(name=f"ps3{rep}", bufs=2, space="PSUM") as ps3,
            tc.tile_pool(name=f"pso{rep}", bufs=1, space="PSUM") as pso,
        ):
            for c in range(NCH):
                csl = slice(c * TC, (c + 1) * TC)
                Bc = bcp.tile([128, N, TC], F32, tag="Bc")
                nc.sync.dma_start(
                    out=Bc[:, :, :],
                    in_=D_["xdbl_dram"][None, RK:RK + N, csl].broadcast_to(
                        [128, N, TC]))
                Cc = bcp.tile([128, N, TC], F32, tag="Cc")
                nc.sync.dma_start(
                    out=Cc[:, :, :],
                    in_=D_["xdbl_dram"][None, RK + N:RK + 2 * N, csl].broadcast_to(
                        [128, N, TC]))
                u3c = bcp.tile([128, NJ, TC], F32, tag="u3c")
                nc.sync.dma_start(
                    out=u3c[:, :, :],
                    in_=D_["u_dram"][:, csl].rearrange("(j p) t -> p j t", j=NJ))

                # dt_proj (bias folded in) + softplus + clip, j-pairs
                dt3 = wkp.tile([128, NJ, TC], F32, tag="dt3")
                for jp in range(NJ // 2):
                    psd = ps3.tile([128, 2, TC], F32, tag="psd")
                    for jj in range(2):
                        j = jp * 2 + jj
                        nc.tensor.matmul(
                            psd[:, jj, :],
                            C["wdtT"][:, j * 128:(j + 1) * 128],
                            dtraw[:, csl], start=True, stop=True)
                    nc.scalar.activation(
                        dt3[:, jp * 2:(jp + 1) * 2, :].rearrange(
                            "p j t -> p (j t)"),
                        psd.rearrange("p j t -> p (j t)"), AF.Exp)
                nc.scalar.activation(
                    dt3.rearrange("p j t -> p (j t)"),
                    dt3.rearrange("p j t -> p (j t)"), AF.Ln, bias=1.0)
                nc.vector.tensor_scalar(
                    dt3.rearrange("p j t -> p (j t)"),
                    dt3.rearrange("p j t -> p (j t)"),
                    1e-4, 20.0, op0=OP.max, op1=OP.min)
                if c == 0 and "dbg_dt" in D_:
                    nc.sync.dma_start(
                        out=D_["dbg_dt"][:, :].rearrange("(j p) t -> p j t",
                                                         j=NJ),
                        in_=dt3[:, :, :])

                yb3 = wkp.tile([128, NJ, TC], F32, tag="yb3")
                for j in range(NJ):
                    a3 = abp.tile([128, N, TC], F32, tag="a3")
                    nc.vector.tensor_tensor(
                        out=a3[:, :, :],
                        in0=dt3[:, j, None, :].broadcast_to([128, N, TC]),
                        in1=C["negA"][:, j, :, None].broadcast_to([128, N, TC]),
                        op=OP.mult)
                    nc.scalar.activation(
                        a3.rearrange("p n t -> p (n t)"),
                        a3.rearrange("p n t -> p (n t)"), AF.Exp)
                    b3 = abp.tile([128, N, TC], F32, tag="b3")
                    nc.vector.tensor_tensor(
                        out=b3[:, :, :],
                        in0=dt3[:, j, None, :].broadcast_to([128, N, TC]),
                        in1=Bc[:, :, :], op=OP.mult)
                    nc.vector.tensor_tensor(
                        out=b3[:, :, :],
                        in0=u3c[:, j, None, :].broadcast_to([128, N, TC]),
                        in1=b3[:, :, :], op=OP.mult)
                    if c > 0:
                        tmp0 = wkp.tile([128, N, 1], F32, tag="tmp0")
                        nc.vector.tensor_tensor(
                            out=tmp0[:, :, :], in0=a3[:, :, 0:1],
                            in1=carries[:, j, :, None], op=OP.mult)
                        nc.vector.tensor_tensor(
                            out=b3[:, :, 0:1], in0=b3[:, :, 0:1],
                            in1=tmp0[:, :, :], op=OP.add)
                    nc.vector.memset(a3[:, :, 0:1], 0.0)
                    nc.vector.tensor_tensor_scan(
                        b3.rearrange("p n t -> p (n t)"),
                        a3.rearrange("p n t -> p (n t)"),
                        b3.rearrange("p n t -> p (n t)"),
                        0.0, op0=OP.mult, op1=OP.add)
                    if c < NCH - 1:
                        nc.vector.tensor_scalar(
                            carries[:, j, :], b3[:, :, TC - 1],
                            0.0, 1.0, op0=OP.add, op1=OP.mult)
                    nc.vector.tensor_tensor(out=a3[:, :, :], in0=b3[:, :, :],
                                            in1=Cc[:, :, :], op=OP.mult)
                    nc.vector.tensor_reduce(
                        out=yb3[:, j, :], in_=a3.rearrange("p n t -> p t n"),
                        axis=mybir.AxisListType.X, op=OP.add)

                # y = clip(y + u*D) * g, whole [128, NJ, TC] at once
                uD = abp.tile([128, NJ, TC], F32, tag="a3", name="uD")
                nc.vector.tensor_tensor(
                    out=uD[:, :, :], in0=u3c[:, :, :],
                    in1=C["dvec"][:, :, None].broadcast_to([128, NJ, TC]),
                    op=OP.mult)
                nc.vector.tensor_tensor(out=yb3[:, :, :], in0=yb3[:, :, :],
                                        in1=uD[:, :, :], op=OP.add)
                nc.vector.tensor_scalar(
                    yb3.rearrange("p j t -> p (j t)"),
                    yb3.rearrange("p j t -> p (j t)"),
                    0.0, 1.0, op0=OP.max, op1=OP.min)
                g3c = abp.tile([128, NJ, TC], F32, tag="b3", name="g3c")
                nc.sync.dma_start(
                    out=g3c[:, :, :],
                    in_=D_["g_dram"][:, csl].rearrange("(j p) t -> p j t", j=NJ))
                nc.vector.tensor_tensor(out=yb3[:, :, :], in0=yb3[:, :, :],
                                        in1=g3c[:, :, :], op=OP.mult)
                if c == 0 and "dbg_yb" in D_:
                    nc.sync.dma_start(
                        out=D_["dbg_yb"][:, :].rearrange("(j p) t -> p j t",
                                                         j=NJ),
                        in_=yb3[:, :, :])

                # out_proj partial over the local d_inner half (2 psum rounds)
                ob = bcp.tile([128, NM, TC], BF16, tag="u3c", name="ob")
                for half in range(2):
                    po = pso.tile([128, NM // 2, TC], F32, tag="po")
                    for mh in range(NM // 2):
                        m = half * (NM // 2) + mh
                        wo = wop.tile([128, NJ, 128], F32, tag="wo")
                        nc.sync.dma_start(
                            out=wo[:, :, :],
                            in_=P["woT"][:, m * 128:(m + 1) * 128].rearrange(
                                "(j p) q -> p j q", j=NJ))
                        for j in range(NJ):
                            nc.tensor.matmul(
                                po[:, mh, :], wo[:, j, :], yb3[:, j, :],
                                start=(j == 0), stop=(j == NJ - 1))
                    nc.vector.tensor_scalar(
                        ob[:, half * (NM // 2):(half + 1) * (NM // 2), :]
                        .rearrange("p m t -> p (m t)"),
                        po.rearrange("p m t -> p (m t)"),
                        0.0, 1.0, op0=OP.add, op1=OP.mult)
                nc.sync.dma_start(
                    out=D_["outp_dram"][c][:, :].rearrange("(m p) t -> p m t",
                                                           p=128),
                    in_=ob[:, :, :])
                nc.gpsimd.collective_compute(
                    "AllReduce", OP.add, replica_groups=GROUPS,
                    ins=[D_["outp_dram"][c][:, :].opt()],
                    outs=[D_["outp_dram"][c][:, :].opt()])

                # reduced chunk -> oslab (collective can't write ExternalOutput)
                fin = bcp.tile([128, NM, TC], BF16, tag="Bc", name="fin")
                nc.sync.dma_start(
                    out=fin[:, :, :],
                    in_=D_["outp_dram"][c][:, :].rearrange("(m p) t -> p m t",
                                                           p=128))
                nc.sync.dma_start(
                    out=P["oslab"][:, csl].rearrange("(m p) t -> p m t", p=128),
                    in_=fin[:, :, :])


def _shard(inputs):
    hs = np.asarray(inputs["hidden_states"], np.float32)
    W_in = np.asarray(inputs["W_in"], np.float32)
    conv_w = np.asarray(inputs["conv_w"], np.float32)
    conv_b = np.asarray(inputs["conv_b"], np.float32)
    W_x = np.asarray(inputs["W_x"], np.float32)
    W_dt = np.asarray(inputs["W_dt"], np.float32)
    b_dt = np.asarray(inputs["b_dt"], np.float32)
    W_out = np.asarray(inputs["W_out"], np.float32)
    A_log = np.asarray(inputs["A_log"], np.float32)
    D = np.asarray(inputs["D"], np.float32)

    negA_full = -np.exp(A_log)                      # (DI, N)
    in_maps = []
    for core in range(8):
        b = core // 2
        h = core % 2
        sl = slice(h * DIH, (h + 1) * DIH)
        m = {
            "hst": np.ascontiguousarray(hs[b].T),
            "wxT": np.ascontiguousarray(W_in[sl].T),
            "wzT": np.ascontiguousarray(W_in[DI + h * DIH:DI + (h + 1) * DIH].T),
            "convw": np.ascontiguousarray(
                conv_w[sl, 0, :].reshape(NJ, 128, KC).transpose(1, 0, 2)),
            "convb": np.ascontiguousarray(conv_b[sl].reshape(NJ, 128).T),
            "wxpT": np.ascontiguousarray(W_x[:, sl].T),
            "wdtT": np.ascontiguousarray(
                np.vstack([W_dt[sl].T, b_dt[sl][None, :]])),
            "negA": np.ascontiguousarray(
                negA_full[sl].reshape(NJ, 128, N).transpose(1, 0, 2)),
            "dvec": np.ascontiguousarray(D[sl].reshape(NJ, 128).T),
            "woT": np.ascontiguousarray(W_out[:, sl].T),
        }
        in_maps.append(m)
    return in_maps


def kernel(**inputs):
    if 1 not in _CACHED_NC:
        _CACHED_NC[1] = _build(1)
    nc = _CACHED_NC[1]
    in_maps = _shard(inputs)
    res = run_bass_kernel_spmd(nc, in_maps, core_ids=list(range(8)))
    out = np.empty((B_, L, DM), np.float32)
    for b in range(B_):
        out[b] = res.results[2 * b]["oslab"].astype(np.float32).T
    return out


# revision 22
# speedup vs baseline: 2.7303x; 1.1403x over previous
"""Mamba block kernel, 8-core tensor-parallel (2 cores per batch, d_inner
split in halves, as in Mamba TP).

Targeting the emulated-NRT backend: per-instruction fixed cost (~40-60us)
dominates and each core's stream executes serially, while separate cores
overlap well on marginal reps. So: minimize per-core instruction count.
Per core (~1.5k instructions vs 3.9k in the 4-core batch-parallel layout):
in_proj/conv/x_proj/dt_proj/scan/out_proj over the local 8 j-tiles
(d_inner half), elementwise work batched into whole-[128, 8*512] tiles,
one pair-AllReduce for x_proj partials and per-chunk bf16 pair-AllReduce
for out_proj partials. dt_proj bias is folded into the matmul via an
extra contraction row so softplus-exp needs no per-j bias and can run on
j-pairs.
"""
import sys
sys.path.insert(0, "/opt/trn_rl_repo")
import numpy as np
import concourse.bass as bass
import concourse.bacc as bacc
import concourse.mybir as mybir
from concourse.tile import TileContext
from concourse.bass_utils import run_bass_kernel_spmd

F32 = mybir.dt.float32
BF16 = mybir.dt.bfloat16
OP = mybir.AluOpType
AF = mybir.ActivationFunctionType

B_, L, DM = 4, 2048, 1024
DI = 2048
DIH = DI // 2         # 1024 per core
N = 16
RK = 64
KC = 4
NJ = DIH // 128       # 8 local d_inner tiles
NK = DM // 128        # 8 k tiles over d_model
NM = DM // 128        # 8 out tiles
TC = 512
NCH = L // TC         # 4 chunks
GROUPS = [[0, 1], [2, 3], [4, 5], [6, 7]]

_CACHED_NC = {}


def _build(reps=1):
    nc = bacc.Bacc(num_devices=8)

    hst = nc.declare_dram_parameter("hst", [DM, L], F32, isOutput=False)
    wxT = nc.declare_dram_parameter("wxT", [DM, DIH], F32, isOutput=False)
    wzT = nc.declare_dram_parameter("wzT", [DM, DIH], F32, isOutput=False)
    convw = nc.declare_dram_parameter("convw", [128, NJ, KC], F32, isOutput=False)
    convb = nc.declare_dram_parameter("convb", [128, NJ], F32, isOutput=False)
    wxpT = nc.declare_dram_parameter("wxpT", [DIH, RK + 2 * N], F32, isOutput=False)
    wdtT = nc.declare_dram_parameter("wdtT", [RK + 1, DIH], F32, isOutput=False)
    negA = nc.declare_dram_parameter("negA", [128, NJ, N], F32, isOutput=False)
    dvec = nc.declare_dram_parameter("dvec", [128, NJ], F32, isOutput=False)
    woT = nc.declare_dram_parameter("woT", [DIH, DM], F32, isOutput=False)
    oslab = nc.declare_dram_parameter("oslab", [DM, L], BF16, isOutput=True)

    P = dict(hst=hst, wxT=wxT, wzT=wzT, convw=convw, convb=convb, wxpT=wxpT,
             wdtT=wdtT, negA=negA, dvec=dvec, woT=woT, oslab=oslab)

    with TileContext(nc) as tc:
        with tc.tile_pool(name="const", bufs=1) as cp:
            C = {}
            C["negA"] = cp.tile([128, NJ, N], F32, tag="negA", name="negA_t")
            nc.sync.dma_start(out=C["negA"][:, :, :], in_=negA[:, :, :])
            C["dvec"] = cp.tile([128, NJ], F32, tag="dvec", name="dvec_t")
            nc.sync.dma_start(out=C["dvec"][:, :], in_=dvec[:, :])
            C["wdtT"] = cp.tile([RK + 1, DIH], F32, tag="wdtT", name="wdtT_t")
            nc.sync.dma_start(out=C["wdtT"][:, :], in_=wdtT[:, :])
            C["convw"] = cp.tile([128, NJ, KC], F32, tag="convw", name="convw_t")
            nc.sync.dma_start(out=C["convw"][:, :, :], in_=convw[:, :, :])
            C["convb"] = cp.tile([128, NJ], F32, tag="convb", name="convb_t")
            nc.sync.dma_start(out=C["convb"][:, :], in_=convb[:, :])

            for rep in range(reps):
                D_ = {
                    "u_dram": nc.dram_tensor(f"u_dram{rep}", [DIH, L], F32),
                    "g_dram": nc.dram_tensor(f"g_dram{rep}", [DIH, L], F32),
                    "xdbl_dram": nc.dram_tensor(f"xdbl_dram{rep}",
                                                [RK + 2 * N, L], F32),
                    "outp_dram": [nc.dram_tensor(f"outp_dram{rep}_{c}",
                                                 [DM, TC], BF16)
                                  for c in range(NCH)],
                }
                _emit(nc, tc, P, C, D_, rep)

    nc.finalize()
    return nc


def _emit(nc, tc, P, C, D_, rep):
    # ================= phase A: in_proj, conv, u/g, x_proj ================
    with tc.tile_pool(name=f"keep{rep}", bufs=1) as kp:
        dtraw = kp.tile([RK + 1, L], F32, tag="dtraw")
        carries = kp.tile([128, NJ, N], F32, tag="carries")

        with tc.tile_pool(name=f"hs{rep}", bufs=1) as hp:
            hs = hp.tile([128, NK, L], F32, tag="hs")
            nc.sync.dma_start(
                out=hs[:, :, :],
                in_=P["hst"][:, :].rearrange("(k p) t -> p k t", k=NK))

            # ---- x half: in_proj + conv + silu + clip -> u; x_proj accum --
            with (
                tc.tile_pool(name=f"w1{rep}", bufs=2) as wp,
                tc.tile_pool(name=f"xc{rep}", bufs=1) as xcp,
                tc.tile_pool(name=f"cv{rep}", bufs=1) as cvp,
                tc.tile_pool(name=f"u1{rep}", bufs=2) as up,
                tc.tile_pool(name=f"psA{rep}", bufs=1, space="PSUM") as psa,
                tc.tile_pool(name=f"psXP{rep}", bufs=1, space="PSUM") as psxp,
            ):
                wxp = xcp.tile([128, NJ, RK + 2 * N], F32, tag="wxp")
                nc.sync.dma_start(
                    out=wxp[:, :, :],
                    in_=P["wxpT"][:, :].rearrange("(j p) w -> p j w", j=NJ))
                xp = psxp.tile([RK + 2 * N, NCH, TC], F32, tag="xp")
                xc = xcp.tile([128, KC - 1 + L], F32, tag="xc")
                nc.vector.memset(xc[:, 0:KC - 1], 0.0)

                for j in range(NJ):
                    wt = wp.tile([128, NK, 128], F32, tag="w_in")
                    nc.sync.dma_start(
                        out=wt[:, :, :],
                        in_=P["wxT"][:, j * 128:(j + 1) * 128].rearrange(
                            "(k p) q -> p k q", k=NK))
                    ps = psa.tile([128, NCH, TC], F32, tag="psA")
                    for c in range(NCH):
                        for k in range(NK):
                            nc.tensor.matmul(
                                ps[:, c, :], wt[:, k, :],
                                hs[:, k, c * TC:(c + 1) * TC],
                                start=(k == 0), stop=(k == NK - 1))
                    nc.vector.tensor_scalar(
                        xc[:, KC - 1:], ps.rearrange("p c t -> p (c t)"),
                        0.0, 1.0, op0=OP.max, op1=OP.min)
                    tp4 = cvp.tile([128, KC, L], F32, tag="tp4")
                    for k in range(KC):
                        nc.vector.tensor_tensor(
                            out=tp4[:, k, :], in0=xc[:, k:k + L],
                            in1=C["convw"][:, j, k:k + 1].broadcast_to([128, L]),
                            op=OP.mult)
                    ca = cvp.tile([128, L], F32, tag="ca")
                    nc.vector.tensor_reduce(
                        out=ca[:, :], in_=tp4.rearrange("p k t -> p t k"),
                        axis=mybir.AxisListType.X, op=OP.add)
                    us = up.tile([128, L], F32, tag="us")
                    nc.scalar.activation(us[:, :], ca[:, :], AF.Silu,
                                         bias=C["convb"][:, j:j + 1])
                    u = up.tile([128, L], F32, tag="u")
                    nc.vector.tensor_scalar(u[:, :], us[:, :], 0.0, 1.0,
                                            op0=OP.max, op1=OP.min)
                    nc.sync.dma_start(
                        out=D_["u_dram"][j * 128:(j + 1) * 128, :], in_=u[:, :])
                    for c in range(NCH):
                        nc.tensor.matmul(
                            xp[:, c, :], wxp[:, j, :], u[:, c * TC:(c + 1) * TC],
                            start=(j == 0), stop=(j == NJ - 1))

                # ---- x_dbl partial out of PSUM -> DRAM -> pair AllReduce --
                xps = up.tile([RK + 2 * N, L], F32, tag="us", name="xps")
                nc.vector.tensor_scalar(
                    xps[:, :], xp.rearrange("p c t -> p (c t)"),
                    0.0, 1.0, op0=OP.add, op1=OP.mult)
                nc.sync.dma_start(out=D_["xdbl_dram"][:, :], in_=xps[:, :])
                nc.gpsimd.collective_compute(
                    "AllReduce", OP.add, replica_groups=GROUPS,
                    ins=[D_["xdbl_dram"][:, :].opt()],
                    outs=[D_["xdbl_dram"][:, :].opt()])

            # ---- z half: in_proj + silu -> g, j-pairs on 8 psum banks ----
            with (
                tc.tile_pool(name=f"w2{rep}", bufs=2) as wp2,
                tc.tile_pool(name=f"zf{rep}", bufs=1) as zfp,
                tc.tile_pool(name=f"psZ{rep}", bufs=1, space="PSUM") as psz,
            ):
                for jp in range(NJ // 2):
                    pz = psz.tile([128, 2, NCH, TC], F32, tag="psZ")
                    for jj in range(2):
                        j = jp * 2 + jj
                        wt = wp2.tile([128, NK, 128], F32, tag="w_in2")
                        nc.sync.dma_start(
                            out=wt[:, :, :],
                            in_=P["wzT"][:, j * 128:(j + 1) * 128].rearrange(
                                "(k p) q -> p k q", k=NK))
                        for c in range(NCH):
                            for k in range(NK):
                                nc.tensor.matmul(
                                    pz[:, jj, c, :], wt[:, k, :],
                                    hs[:, k, c * TC:(c + 1) * TC],
                                    start=(k == 0), stop=(k == NK - 1))
                    zf = zfp.tile([128, 2, L], F32, tag="zf")
                    nc.vector.tensor_scalar(
                        zf.rearrange("p j t -> p (j t)"),
                        pz.rearrange("p j c t -> p (j c t)"),
                        0.0, 1.0, op0=OP.max, op1=OP.min)
                    g2 = zfp.tile([128, 2, L], F32, tag="g2")
                    nc.scalar.activation(
                        g2.rearrange("p j t -> p (j t)"),
                        zf.rearrange("p j t -> p (j t)"), AF.Silu)
                    nc.sync.dma_start(
                        out=D_["g_dram"][jp * 256:(jp + 1) * 256, :].rearrange(
                            "(j p) t -> p j t", j=2),
                        in_=g2[:, :, :])

            # ---- dtraw = clip(x_dbl[0:RK]) after the AllReduce; ones row --
            nc.sync.dma_start(out=dtraw[0:RK, :], in_=D_["xdbl_dram"][0:RK, :])
            nc.vector.tensor_scalar(dtraw[0:RK, :], dtraw[0:RK, :], 0.0, 1.0,
                                    op0=OP.max, op1=OP.min)
            nc.vector.memset(dtraw[RK:RK + 1, :], 1.0)

        # ================= phase B: dt, scan, gate, out_proj ==============
        with (
            tc.tile_pool(name=f"bc{rep}", bufs=1) as bcp,
            tc.tile_pool(name=f"ab{rep}", bufs=1) as abp,
            tc.tile_pool(name=f"wk{rep}", bufs=1) as wkp,
            tc.tile_pool(name=f"wo{rep}", bufs=1) as wop,
            tc.tile_pool(name=f"ps3{rep}", bufs=2, space="PSUM") as ps3,
            tc.tile_pool(name=f"pso{rep}", bufs=1, space="PSUM") as pso,
        ):
            woa = wop.tile([128, NJ, NM, 128], F32, tag="woa")
            nc.sync.dma_start(
                out=woa[:, :, :, :],
                in_=P["woT"][:, :].rearrange("(j p) (m q) -> p j m q",
                                             j=NJ, m=NM))
            for c in range(NCH):
                csl = slice(c * TC, (c + 1) * TC)
                BCc = bcp.tile([128, 2 * N, TC], BF16, tag="BCc")
                nc.gpsimd.dma_start(
                    out=BCc[:, :, :],
                    in_=D_["xdbl_dram"][None, RK:RK + 2 * N, csl].broadcast_to(
                        [128, 2 * N, TC]))
                Bc = BCc[:, 0:N, :]
                Cc = BCc[:, N:2 * N, :]
                u3c = bcp.tile([128, NJ, TC], F32, tag="u3c")
                nc.sync.dma_start(
                    out=u3c[:, :, :],
                    in_=D_["u_dram"][:, csl].rearrange("(j p) t -> p j t", j=NJ))

                # dt_proj (bias folded in) + softplus + clip, j-pairs
                dt3 = wkp.tile([128, NJ, TC], F32, tag="dt3")
                for jp in range(NJ // 2):
                    psd = ps3.tile([128, 2, TC], F32, tag="psd")
                    for jj in range(2):
                        j = jp * 2 + jj
                        nc.tensor.matmul(
                            psd[:, jj, :],
                            C["wdtT"][:, j * 128:(j + 1) * 128],
                            dtraw[:, csl], start=True, stop=True)
                    nc.scalar.activation(
                        dt3[:, jp * 2:(jp + 1) * 2, :].rearrange(
                            "p j t -> p (j t)"),
                        psd.rearrange("p j t -> p (j t)"), AF.Exp)
                nc.scalar.activation(
                    dt3.rearrange("p j t -> p (j t)"),
                    dt3.rearrange("p j t -> p (j t)"), AF.Ln, bias=1.0)
                nc.vector.tensor_scalar(
                    dt3.rearrange("p j t -> p (j t)"),
                    dt3.rearrange("p j t -> p (j t)"),
                    1e-4, 20.0, op0=OP.max, op1=OP.min)
                if c == 0 and "dbg_dt" in D_:
                    nc.sync.dma_start(
                        out=D_["dbg_dt"][:, :].rearrange("(j p) t -> p j t",
                                                         j=NJ),
                        in_=dt3[:, :, :])

                yb3 = wkp.tile([128, NJ, TC], F32, tag="yb3")
                for j in range(NJ):
                    a3 = abp.tile([128, N, TC], F32, tag="a3")
                    nc.vector.tensor_tensor(
                        out=a3[:, :, :],
                        in0=dt3[:, j, None, :].broadcast_to([128, N, TC]),
                        in1=C["negA"][:, j, :, None].broadcast_to([128, N, TC]),
                        op=OP.mult)
                    nc.scalar.activation(
                        a3.rearrange("p n t -> p (n t)"),
                        a3.rearrange("p n t -> p (n t)"), AF.Exp)
                    b3 = abp.tile([128, N, TC], F32, tag="b3")
                    nc.vector.tensor_tensor(
                        out=b3[:, :, :],
                        in0=dt3[:, j, None, :].broadcast_to([128, N, TC]),
                        in1=Bc[:, :, :], op=OP.mult)
                    nc.vector.tensor_tensor(
                        out=b3[:, :, :],
                        in0=u3c[:, j, None, :].broadcast_to([128, N, TC]),
                        in1=b3[:, :, :], op=OP.mult)
                    if c > 0:
                        tmp0 = wkp.tile([128, N, 1], F32, tag="tmp0")
                        nc.vector.tensor_tensor(
                            out=tmp0[:, :, :], in0=a3[:, :, 0:1],
                            in1=carries[:, j, :, None], op=OP.mult)
                        nc.vector.tensor_tensor(
                            out=b3[:, :, 0:1], in0=b3[:, :, 0:1],
                            in1=tmp0[:, :, :], op=OP.add)
                    nc.vector.memset(a3[:, :, 0:1], 0.0)
                    nc.vector.tensor_tensor_scan(
                        b3.rearrange("p n t -> p (n t)"),
                        a3.rearrange("p n t -> p (n t)"),
                        b3.rearrange("p n t -> p (n t)"),
                        0.0, op0=OP.mult, op1=OP.add)
                    if c < NCH - 1:
                        nc.vector.tensor_scalar(
                            carries[:, j, :], b3[:, :, TC - 1],
                            0.0, 1.0, op0=OP.add, op1=OP.mult)
                    nc.vector.tensor_tensor(out=a3[:, :, :], in0=b3[:, :, :],
                                            in1=Cc[:, :, :], op=OP.mult)
                    nc.vector.tensor_reduce(
                        out=yb3[:, j, :], in_=a3.rearrange("p n t -> p t n"),
                        axis=mybir.AxisListType.X, op=OP.add)

                # y = clip(y + u*D) * g, whole [128, NJ, TC] at once
                uD = abp.tile([128, NJ, TC], F32, tag="a3", name="uD")
                nc.vector.tensor_tensor(
                    out=uD[:, :, :], in0=u3c[:, :, :],
                    in1=C["dvec"][:, :, None].broadcast_to([128, NJ, TC]),
                    op=OP.mult)
                nc.vector.tensor_tensor(out=yb3[:, :, :], in0=yb3[:, :, :],
                                        in1=uD[:, :, :], op=OP.add)
                nc.vector.tensor_scalar(
                    yb3.rearrange("p j t -> p (j t)"),
                    yb3.rearrange("p j t -> p (j t)"),
                    0.0, 1.0, op0=OP.max, op1=OP.min)
                g3c = abp.tile([128, NJ, TC], F32, tag="b3", name="g3c")
                nc.sync.dma_start(
                    out=g3c[:, :, :],
                    in_=D_["g_dram"][:, csl].rearrange("(j p) t -> p j t", j=NJ))
                nc.vector.tensor_tensor(out=yb3[:, :, :], in0=yb3[:, :, :],
                                        in1=g3c[:, :, :], op=OP.mult)
                if c == 0 and "dbg_yb" in D_:
                    nc.sync.dma_start(
                        out=D_["dbg_yb"][:, :].rearrange("(j p) t -> p j t",
                                                         j=NJ),
                        in_=yb3[:, :, :])

                # out_proj partial over the local d_inner half (2 psum rounds)
                ob = bcp.tile([128, NM, TC], BF16, tag="u3c", name="ob")
                for half in range(2):
                    po = pso.tile([128, NM // 2, TC], F32, tag="po")
                    for mh in range(NM // 2):
                        m = half * (NM // 2) + mh
                        for j in range(NJ):
                            nc.tensor.matmul(
                                po[:, mh, :], woa[:, j, m, :], yb3[:, j, :],
                                start=(j == 0), stop=(j == NJ - 1))
                    nc.vector.tensor_scalar(
                        ob[:, half * (NM // 2):(half + 1) * (NM // 2), :]
                        .rearrange("p m t -> p (m t)"),
                        po.rearrange("p m t -> p (m t)"),
                        0.0, 1.0, op0=OP.add, op1=OP.mult)
                nc.sync.dma_start(
                    out=D_["outp_dram"][c][:, :].rearrange("(m p) t -> p m t",
                                                           p=128),
                    in_=ob[:, :, :])
                nc.gpsimd.collective_compute(
                    "AllReduce", OP.add, replica_groups=GROUPS,
                    ins=[D_["outp_dram"][c][:, :].opt()],
                    outs=[D_["outp_dram"][c][:, :].opt()])

                # reduced chunk -> oslab (collective can't write ExternalOutput)
                fin = bcp.tile([128, NM, TC], BF16, tag="BCc", name="fin")
                nc.sync.dma_start(
                    out=fin[:, :, :],
                    in_=D_["outp_dram"][c][:, :].rearrange("(m p) t -> p m t",
                                                           p=128))
                nc.sync.dma_start(
                    out=P["oslab"][:, csl].rearrange("(m p) t -> p m t", p=128),
                    in_=fin[:, :, :])


def _shard(inputs):
    hs = np.asarray(inputs["hidden_states"], np.float32)
    W_in = np.asarray(inputs["W_in"], np.float32)
    conv_w = np.asarray(inputs["conv_w"], np.float32)
    conv_b = np.asarray(inputs["conv_b"], np.float32)
    W_x = np.asarray(inputs["W_x"], np.float32)
    W_dt = np.asarray(inputs["W_dt"], np.float32)
    b_dt = np.asarray(inputs["b_dt"], np.float32)
    W_out = np.asarray(inputs["W_out"], np.float32)
    A_log = np.asarray(inputs["A_log"], np.float32)
    D = np.asarray(inputs["D"], np.float32)

    negA_full = -np.exp(A_log)                      # (DI, N)
    in_maps = []
    for core in range(8):
        b = core // 2
        h = core % 2
        sl = slice(h * DIH, (h + 1) * DIH)
        m = {
            "hst": np.ascontiguousarray(hs[b].T),
            "wxT": np.ascontiguousarray(W_in[sl].T),
            "wzT": np.ascontiguousarray(W_in[DI + h * DIH:DI + (h + 1) * DIH].T),
            "convw": np.ascontiguousarray(
                conv_w[sl, 0, :].reshape(NJ, 128, KC).transpose(1, 0, 2)),
            "convb": np.ascontiguousarray(conv_b[sl].reshape(NJ, 128).T),
            "wxpT": np.ascontiguousarray(W_x[:, sl].T),
            "wdtT": np.ascontiguousarray(
                np.vstack([W_dt[sl].T, b_dt[sl][None, :]])),
            "negA": np.ascontiguousarray(
                negA_full[sl].reshape(NJ, 128, N).transpose(1, 0, 2)),
            "dvec": np.ascontiguousarray(D[sl].reshape(NJ, 128).T),
            "woT": np.ascontiguousarray(W_out[:, sl].T),
        }
        in_maps.append(m)
    return in_maps


def kernel(**inputs):
    if 1 not in _CACHED_NC:
        _CACHED_NC[1] = _build(1)
    nc = _CACHED_NC[1]
    in_maps = _shard(inputs)
    res = run_bass_kernel_spmd(nc, in_maps, core_ids=list(range(8)))
    out = np.empty((B_, L, DM), np.float32)
    for b in range(B_):
        out[b] = res.results[2 * b]["oslab"].astype(np.float32).T
    return out
